# revision 54
# baseline (speedup 1.0000x reference)
"""Trainium2 Bass kernel for nn_Attention_89902255440825.

Single-layer attention block: QKV proj + per-head RMS("mult" variant) +
RoPE + GQA causal attention with softmax(scores * sqrt(HD)) + O proj.

Sharding (8 NeuronCores, tensor-parallel over heads):
  core c: q heads {2c, 2c+1}  (wq cols 256c:256c+256)
          kv head c//2        (wk/wv cols 128*(c//2):...)
          wo rows 256c:256c+256  -> partial [S,H] outputs, summed on host.

Precision strategy (measured on HW):
  - q/k path (projections + scores) splits each operand into an fp16 hi
    term (1 matmul, 1 cyc/row) plus both hi*lo cross terms packed into
    ONE fp8e4m3 DoubleRow matmul (0.5 cyc/row): 1.5 cyc/row total vs 3
    for an fp16 hi/lo triple, at logit error ~5e-3 absmax (HW-measured).
    Power-of-2 scales keep every fp8 operand in e4m3's normal range and
    are folded into host-preprocessed weights, the rope tables, the
    projection bias step, and the softmax exp scale.
  - v / PV / O-proj use single-pass fp16 (error ~1e-3, benign).
  - query-side RMS scale s_q applied inside exp (per-partition scale AP).
  - causal mask added on the PE (bf16 identity @ bf16 -1e30 mask) as part
    of the scores PSUM accumulation; diag chunks trimmed to (i%4+1)*128.

Schedule: projection quarters and attention i-blocks are interleaved
(attention batch m needs only k/q quarters <= m), and attention is
software-pipelined (scores of block i+1 are emitted before the
exp-dependent stage of block i) so the in-order PE queue never waits
on the softmax latency chain.
"""
import numpy as np
import ml_dtypes
from contextlib import ExitStack

import concourse.bass as bass
import concourse.tile as tile
from concourse import bacc, mybir, bass_utils
from concourse.masks import make_identity

S = 2048
H = 2048
HD = 128
NH = 16
NKV = 4
NCORES = 8
HPC = NH // NCORES          # q heads per core = 2
FQ = HPC * HD               # q features per core = 256
EPS = 1e-6
NEG = -1e30
F32 = mybir.dt.float32
F16 = mybir.dt.float16
BF16 = mybir.dt.bfloat16
F8 = mybir.dt.float8e4
E4M3 = ml_dtypes.float8_e4m3
AX = mybir.AxisListType.X
AXY = mybir.AxisListType.XY
OP = mybir.AluOpType
ACTF = mybir.ActivationFunctionType
DR = mybir.MatmulPerfMode.DoubleRow

# power-of-2 scales
SC_X = 64.0        # x hi fp16 = fp16(2^6 x)
SC_W = 2048.0      # w hi fp16 = fp16(2^11 w); proj psum = 2^17 * value
INV_PROJ = 1.0 / (SC_X * SC_W)      # 2^-17
SC_WL = float(2 ** 17)   # w-lo fp8 scale (pairs with x-hi fp8)
SC_XL = 2048.0           # x-lo fp8 scale (pairs with w-hi fp8 = 2^6 w)
SC_WH8 = 64.0            # w-hi fp8 scale
SC_R = 32.0        # rope tables carry 2^5; scores psum = 2^10 * logits/s_q
INV_SCORE = 1.0 / (SC_R * SC_R)     # 2^-10

_prog_cache = {}


def _build(is_causal: bool):
    nc = bacc.Bacc("TRN2", target_bir_lowering=False, debug=False,
                   num_devices=NCORES)

    def din(name, shape, dt=F16):
        return nc.dram_tensor(name, shape, dt, kind="ExternalInput").ap()

    # x operands pre-swizzled on host: [group, quarter, p, ...] so each
    # 4-kblock group loads as one flat [128, 2048/4096] DMA
    xh_d = din("xh", [4, 4, 128, 4 * 512])          # fp16(2^6 xT) grouped
    xdr_d = din("xdr", [4, 4, 128, 4 * 2 * 512], F8)  # fp8 DR pairs grouped
    wqh_d = din("wqh", [H, FQ])               # fp16(2^11 wq)
    wqdr_d = din("wqdr", [H, 2, FQ], F8)      # [e4m3(2^17 wl), e4m3(2^6 w)]
    wkh_d = din("wkh", [H, HD])
    wkdr_d = din("wkdr", [H, 2, HD], F8)
    wvh_d = din("wvh", [H, HD])               # fp16(2^11 wv)
    woh_d = din("woh", [FQ, H])               # fp16(wo)
    cosq_d = din("cosq", [HD, S], F32)        # 2^5 cos * qn
    sinq_d = din("sinq", [HD, S], F32)        # 2^5 sin * roll(qn), half neg
    cosk_d = din("cosk", [HD, S], F32)
    sink_d = din("sink", [HD, S], F32)
    bqt_d = din("bqt", [HD, HPC], F32)
    bkt_d = din("bkt", [HD, 1], F32)
    bvt_d = din("bvt", [HD, 1], F32)
    if not is_causal:
        mask_d = din("maskadd", [S, S], F32)
    out_d = nc.dram_tensor("out", [S, H], F16, kind="ExternalOutput").ap()

    NKB = H // 128            # 16 contraction k-blocks
    NMB = S // 128            # 16 token blocks
    NCH = S // 512            # 4 512-chunks

    with tile.TileContext(nc) as tc, ExitStack() as ctx:
        const = ctx.enter_context(tc.tile_pool(name="const", bufs=1))
        wpool = ctx.enter_context(tc.tile_pool(name="wpool", bufs=1))
        big = ctx.enter_context(tc.tile_pool(name="big", bufs=1))
        xpool = ctx.enter_context(tc.tile_pool(name="xpool", bufs=2))
        btmp = ctx.enter_context(tc.tile_pool(name="btmp", bufs=1))
        cpool = ctx.enter_context(tc.tile_pool(name="cpool", bufs=2))
        dpool = ctx.enter_context(tc.tile_pool(name="dpool", bufs=2))
        psum = ctx.enter_context(tc.tile_pool(name="psum", bufs=1, space="PSUM"))

        # ---- constants ----
        ident16 = const.tile([128, 128], F16)
        make_identity(nc, ident16[:])
        identb = const.tile([128, 128], BF16)
        make_identity(nc, identb[:])
        ones16 = const.tile([128, 1], F16)
        nc.vector.memset(ones16[:], 1.0)
        ones11 = const.tile([1, 1], F32)
        nc.vector.memset(ones11[:], 1.0)
        eps_q = const.tile([1, 1], F32)   # sqrt(x + 128*eps) = 11.31*sqrt(x/128+eps)
        nc.vector.memset(eps_q[:], EPS * HD)
        eps_k = const.tile([1, 1], F32)
        nc.vector.memset(eps_k[:], EPS)
        if is_causal:
            # bf16 -1e30 mask blocks, added to scores on the PE via identb
            cmask = const.tile([128, 4, 512], BF16)
            for r in range(4):
                wr = (r + 1) * 128
                nc.vector.memset(cmask[:, r, 0:wr], 0.0)
                nc.gpsimd.affine_select(
                    out=cmask[:, r, 0:wr], in_=cmask[:, r, 0:wr],
                    compare_op=OP.is_ge, fill=NEG,
                    base=128 * r, channel_multiplier=1, pattern=[[-1, wr]],
                )

        # ---- weights / small inputs to SBUF ----
        wqh_sb = wpool.tile([128, NKB, FQ], F16)
        wqdr_sb = wpool.tile([128, NKB, 2, FQ], F8)
        wkh_sb = wpool.tile([128, NKB, HD], F16)
        wkdr_sb = wpool.tile([128, NKB, 2, HD], F8)
        wvh_sb = wpool.tile([128, NKB, HD], F16)

        def wload(grp, t0=None, t1=None):
            if t0 is None:
                t0, t1 = grp * 4, (grp + 1) * 4
            rs = slice(t0 * 128, t1 * 128)
            nc.sync.dma_start(
                wqh_sb[:, t0:t1], wqh_d[rs].rearrange("(t p) f -> p t f", p=128))
            nc.sync.dma_start(
                wqdr_sb[:, t0:t1],
                wqdr_d[rs].rearrange("(t p) two f -> p t two f", p=128))
            nc.sync.dma_start(
                wkh_sb[:, t0:t1], wkh_d[rs].rearrange("(t p) f -> p t f", p=128))
            nc.sync.dma_start(
                wkdr_sb[:, t0:t1],
                wkdr_d[rs].rearrange("(t p) two f -> p t two f", p=128))
            nc.sync.dma_start(
                wvh_sb[:, t0:t1], wvh_d[rs].rearrange("(t p) f -> p t f", p=128))

        woh_sb = wpool.tile([128, HPC, H], F16)
        bqt_sb = wpool.tile([HD, HPC], F32)
        nc.sync.dma_start(bqt_sb[:], bqt_d)
        bkt_sb = wpool.tile([HD, 1], F32)
        nc.sync.dma_start(bkt_sb[:], bkt_d)
        bvt_sb = wpool.tile([HD, 1], F32)
        nc.sync.dma_start(bvt_sb[:], bvt_d)

        # ---- persistent activations ----
        vt16 = big.tile([128, S], F16)            # v feature-major fp16
        v_sb = big.tile([128, NMB, 128], F16)     # v token-major fp16
        qh16 = big.tile([128, HPC, S], F16)       # roped q hi (2^5 scale)
        qdr8 = big.tile([128, HPC, 2, S], F8)     # [e4m3(2^10 ql), e4m3(q)]
        kh16 = big.tile([128, S], F16)
        kdr8 = big.tile([128, 2, S], F8)          # [e4m3(k), e4m3(2^10 kl)]
        sqs = big.tile([1, HPC, S], F32)          # q RMS scale rows

        # ================= phase A+B for one quarter =================
        def quarter(mq_):
            ms = slice(mq_ * 512, (mq_ + 1) * 512)
            if mq_ == 0:
                wload(0, 0, 2)
            # rope-table slices for this quarter's phase B (ring tiles)
            cosk_sb = btmp.tile([HD, 512], F32, tag="cosk", bufs=2,
                                name=f"cosk_{mq_}")
            sink_sb = btmp.tile([HD, 512], F32, tag="sink", bufs=2,
                                name=f"sink_{mq_}")
            cosq_sb = btmp.tile([HD, 512], F32, tag="cosq", bufs=2,
                                name=f"cosq_{mq_}")
            sinq_sb = btmp.tile([HD, 512], F32, tag="sinq", bufs=2,
                                name=f"sinq_{mq_}")
            nc.sync.dma_start(cosk_sb[:], cosk_d[:, ms])
            nc.sync.dma_start(sink_sb[:], sink_d[:, ms])
            nc.sync.dma_start(cosq_sb[:], cosq_d[:, ms])
            nc.sync.dma_start(sinq_sb[:], sinq_d[:, ms])
            pq = [psum.tile([128, 512], F32, tag="t512", bufs=4,
                            name=f"pq{fb}_{mq_}") for fb in range(HPC)]
            pk = psum.tile([128, 512], F32, tag="t512", bufs=4,
                           name=f"pk_{mq_}")
            pv = psum.tile([128, 512], F32, tag="t512", bufs=4,
                           name=f"pv_{mq_}")
            for g in range(4):            # 4 k-blocks per x DMA group
                kb0 = g * 4
                xh_t = xpool.tile([128, 4, 512], F16, tag="xh",
                                  name=f"xh_{mq_}_{g}")
                nc.sync.dma_start(xh_t[:], xh_d[g, mq_])
                xdr_t = xpool.tile([128, 4, 2, 512], F8, tag="xdr",
                                   name=f"xdr_{mq_}_{g}")
                nc.sync.dma_start(xdr_t[:], xdr_d[g, mq_])
                if mq_ == 0 and g == 0:
                    wload(0, 2, 4)
                if mq_ == 0 and g < 3:
                    wload(g + 1)
                for j in range(4):
                    kb = kb0 + j
                    st = kb == 0
                    sp = kb == NKB - 1
                    for fb in range(HPC):
                        fsl = slice(fb * 128, (fb + 1) * 128)
                        nc.tensor.matmul(pq[fb][:], wqh_sb[:, kb, fsl],
                                         xh_t[:, j], start=st, stop=False)
                        nc.tensor.matmul(pq[fb][:], wqdr_sb[:, kb, :, fsl],
                                         xdr_t[:, j], start=False, stop=sp,
                                         perf_mode=DR)
                    nc.tensor.matmul(pk[:], wkh_sb[:, kb, :], xh_t[:, j],
                                     start=st, stop=False)
                    nc.tensor.matmul(pk[:], wkdr_sb[:, kb, :, :], xdr_t[:, j],
                                     start=False, stop=sp, perf_mode=DR)
                    nc.tensor.matmul(pv[:], wvh_sb[:, kb, :], xh_t[:, j],
                                     start=st, stop=sp)
            qkt = btmp.tile([128, 3, 512], F32, tag="qkt", bufs=1,
                            name=f"qkt_{mq_}")
            qt_q = qkt[:, 0:HPC, :]
            kt_q = qkt[:, HPC, :]
            for fb in range(HPC):
                nc.vector.tensor_scalar(qt_q[:, fb, :], pq[fb][:],
                                        INV_PROJ, bqt_sb[:, fb:fb + 1],
                                        OP.mult, OP.add)
            nc.vector.tensor_scalar(kt_q, pk[:], INV_PROJ, bkt_sb[:],
                                    OP.mult, OP.add)
            nc.vector.tensor_scalar(vt16[:, ms], pv[:], INV_PROJ, bvt_sb[:],
                                    OP.mult, OP.add)

            if mq_ == 1:
                nc.sync.dma_start(
                    woh_sb[:], woh_d.rearrange("(t p) f -> p t f", p=128))

            # ---- phase B: RMS + RoPE + hi/fp8 split ----
            specs = [
                (kt_q, kh16[:, ms], kdr8[:, :, ms], eps_k, 1.0 / HD,
                 cosk_sb, sink_sb, None),
                (qt_q[:, 0], qh16[:, 0, ms], qdr8[:, 0, :, ms], eps_q, 1.0,
                 cosq_sb, sinq_sb, sqs[:, 0, ms]),
                (qt_q[:, 1], qh16[:, 1, ms], qdr8[:, 1, :, ms], eps_q, 1.0,
                 cosq_sb, sinq_sb, sqs[:, 1, ms]),
            ]
            for sp_i, (bsrc, dsth, dstdr, epst, sscale, cos_sb, sin_sb,
                       sq_dst) in enumerate(specs):
                sq16 = btmp.tile([128, 512], F16, tag="sq", bufs=1,
                                 name=f"sq_{mq_}_{sp_i}")
                nc.gpsimd.tensor_mul(sq16[:], bsrc, bsrc)
                pss = psum.tile([1, 512], F32, tag="pod", bufs=1,
                                name=f"pss_{mq_}_{sp_i}")
                nc.tensor.matmul(pss[:], ones16[:], sq16[:],
                                 start=True, stop=True)
                if sq_dst is None:
                    ssb = btmp.tile([1, 512], F32, tag="ssb", bufs=2)
                    sdst = ssb[:]
                else:
                    sdst = sq_dst
                nc.scalar.activation(sdst, pss[:], ACTF.Sqrt,
                                     bias=epst[:], scale=sscale)
                if sq_dst is None:
                    # k: apply RMS scale now via partition broadcast
                    sbc = btmp.tile([128, 512], F32, tag="sbc", bufs=2)
                    nc.gpsimd.partition_broadcast(sbc[:], sdst)
                    t1 = btmp.tile([128, 512], F32, tag="t1", bufs=2)
                    nc.gpsimd.tensor_mul(t1[:], bsrc, sbc[:])
                    rsrc = t1[:]
                else:
                    # q: RMS scale folded into the exp (scale AP)
                    rsrc = bsrc
                # rope: z = rsrc*cos + swap(rsrc)*sin_eff  (2^5 in tables)
                bt = btmp.tile([128, 3, 512], F32, tag="bt", bufs=2,
                               name=f"bt_{mq_}_{sp_i}")
                u = bt[:, 0, :]
                v = bt[:, 1, :]
                z = bt[:, 2, :]
                nc.vector.tensor_mul(u, rsrc, cos_sb[:])
                # sin tables are host-rotated by 64 rows so both inputs
                # share a base partition (HW constraint on 2-input DVE ops)
                nc.vector.tensor_mul(v[0:64], rsrc[64:128, :],
                                     sin_sb[64:128, :])
                nc.vector.tensor_mul(v[64:128], rsrc[0:64, :],
                                     sin_sb[0:64, :])
                nc.vector.tensor_add(z, u, v)
                # splits: hi fp16 (keeps 2^5), fp8 slots. DR pairing sums
                # slot-wise: q packs (lo, hi), k packs (hi, lo).
                hi_slot, lo_slot = (1, 0) if sq_dst is not None else (0, 1)
                nc.gpsimd.tensor_copy(dsth, z)
                nc.gpsimd.tensor_scalar_mul(dstdr[:, hi_slot], z, 1.0 / SC_R)
                zl = u      # reuse the u slice as the lo-residual temp
                nc.vector.tensor_sub(zl, z, dsth)
                nc.gpsimd.tensor_scalar_mul(dstdr[:, lo_slot], zl, SC_R)

            # v quarter blocks: feature-major -> token-major via PE transpose
            for mb in range(mq_ * 4, (mq_ + 1) * 4):
                pvt = psum.tile([128, 128], F16, tag="t128", bufs=3,
                                name=f"pvt_{mb}")
                nc.tensor.transpose(pvt[:], vt16[:, mb * 128:(mb + 1) * 128],
                                    ident16[:])
                nc.scalar.copy(v_sb[:, mb], pvt[:])

        # ================= attention stage 1: scores + row maxes ==========
        def stage1(i):
            nfull = i // 4 if is_causal else NCH      # full 512-key chunks
            dw = (i % 4 + 1) * 128                    # trimmed diag width
            nchunks = nfull + (1 if is_causal else 0)

            scols, s_sbs, biass = [], [], []
            for h in range(HPC):
                ps_sc = psum.tile([128, 1], F32, tag="t128", bufs=3,
                                  name=f"ps_sc_{i}_{h}")
                nc.tensor.matmul(
                    ps_sc[:], sqs[:, h, i * 128:(i + 1) * 128],
                    ones11[:], start=True, stop=True)
                scol = cpool.tile([128, 1], F32, tag="scol", bufs=4,
                                  name=f"scol_{i}_{h}")
                nc.vector.tensor_scalar_mul(scol[:], ps_sc[:], INV_SCORE)
                scols.append(scol)

            for h in range(HPC):
                qh_blk = qh16[:, h, i * 128:(i + 1) * 128]
                qdr_blk = qdr8[:, h, :, i * 128:(i + 1) * 128]
                s_sb = cpool.tile([128, NCH, 512], F32, tag="s_sb",
                                  bufs=4, name=f"s_sb_{i}_{h}")
                # scores psum (mask folded in on the PE for the diag
                # chunk) -> sbuf copy; row max via Pool element-wise
                # accumulation + one DVE reduce
                mxs = cpool.tile([128, NCH], F32, tag="mxs", bufs=2,
                                 name=f"mxs_{i}_{h}")
                for ncj in range(nchunks):
                    diag = is_causal and ncj == nfull
                    w = dw if diag else 512
                    ks = slice(ncj * 512, ncj * 512 + w)
                    ps_s = psum.tile([128, 512], F32, tag="t512", bufs=4,
                                     name=f"ps_s_{i}_{h}_{ncj}")
                    nc.tensor.matmul(ps_s[:, 0:w], qh_blk, kh16[:, ks],
                                     start=True, stop=False)
                    nc.tensor.matmul(ps_s[:, 0:w], qdr_blk, kdr8[:, :, ks],
                                     start=False, stop=not diag,
                                     perf_mode=DR)
                    if diag:
                        nc.tensor.matmul(ps_s[:, 0:w], identb[:],
                                         cmask[:, i % 4, 0:w],
                                         start=False, stop=True)
                    if not is_causal:
                        madd = cpool.tile([128, 512], F32, tag="mload",
                                          bufs=3, name=f"mload_{i}_{h}_{ncj}")
                        nc.sync.dma_start(
                            madd[:], mask_d[i * 128:(i + 1) * 128, ks])
                        nc.vector.tensor_add(s_sb[:, ncj, 0:w], ps_s[:, 0:w],
                                             madd[:, 0:w])
                    elif ncj == 0 and nchunks > 1:
                        nc.scalar.copy(s_sb[:, ncj, 0:w], ps_s[:, 0:w])
                    else:
                        nc.vector.tensor_copy(s_sb[:, ncj, 0:w], ps_s[:, 0:w])
                mx = cpool.tile([128, 1], F32, tag="mx", bufs=4,
                                name=f"mx_{i}_{h}")
                if nchunks > 1:
                    for ncj in range(nchunks):
                        w = dw if (is_causal and ncj == nfull) else 512
                        nc.vector.reduce_max(mxs[:, ncj:ncj + 1],
                                             s_sb[:, ncj, 0:w], axis=AX)
                    nc.vector.reduce_max(mx[:], mxs[:, 0:nchunks], axis=AX)
                else:
                    nc.vector.reduce_max(mx[:], s_sb[:, 0, 0:dw], axis=AX)
                # exp bias = -s_col * rowmax  (per-partition)
                bias = cpool.tile([128, 1], F32, tag="bias", bufs=4,
                                  name=f"bias_{i}_{h}")
                nc.vector.tensor_scalar(bias[:], mx[:],
                                        scols[h][:], -1.0, OP.mult, OP.mult)
                s_sbs.append(s_sb)
                biass.append(bias)
            return dict(i=i, nfull=nfull, dw=dw, nchunks=nchunks,
                        scols=scols, s_sbs=s_sbs, biass=biass)

        # ====== attention stage 2: exp -> transpose -> PV -> O proj =======
        def stage2(st):
            i = st["i"]
            nfull, dw, nchunks = st["nfull"], st["dw"], st["nchunks"]
            last_nkb = i if is_causal else NCH * 4 - 1
            p16s, lpartss, nparts = [], [], []
            for h in range(HPC):
                s_sb, bias, scol = st["s_sbs"][h], st["biass"][h], st["scols"][h]
                p16 = cpool.tile([128, NCH, 512], F16, tag="p16", bufs=3,
                                 name=f"p16_{i}_{h}")
                lparts = cpool.tile([128, 3], F32, tag="lparts", bufs=4,
                                    name=f"lparts_{i}_{h}")
                npart = 0
                if nfull > 0:        # all full chunks in one instruction
                    nc.scalar.activation(
                        p16[:, 0:nfull, :], s_sb[:, 0:nfull, :],
                        ACTF.Exp, bias=bias[:], scale=scol[:],
                        accum_out=lparts[:, npart:npart + 1])
                    npart += 1
                if is_causal:        # trimmed diag chunk
                    nc.scalar.activation(
                        p16[:, nfull, 0:dw], s_sb[:, nfull, 0:dw],
                        ACTF.Exp, bias=bias[:], scale=scol[:],
                        accum_out=lparts[:, npart:npart + 1])
                    npart += 1
                p16s.append(p16)
                lpartss.append(lparts)
                nparts.append(npart)

            out_ps2 = psum.tile([128, HPC, 128], F32, tag="t128",
                                bufs=3, name=f"out_ps2_{i}")
            out_pss, linvs = [], []
            for h in range(HPC):
                p16 = p16s[h]
                out_ps = out_ps2[:, h, :]
                for ncj in range(nchunks):
                    nb = min(4, last_nkb + 1 - ncj * 4)
                    ps_t4 = psum.tile([128, 4, 128], F16, tag="t128", bufs=3,
                                      name=f"ps_t4_{i}_{h}_{ncj}")
                    for b in range(nb):
                        nc.tensor.transpose(
                            ps_t4[:, b, :],
                            p16[:, ncj, b * 128:(b + 1) * 128],
                            ident16[:])
                    pt_sb = cpool.tile([128, 4, 128], F16, tag="pt_sb",
                                       bufs=3, name=f"pt_sb_{i}_{h}_{ncj}")
                    if ncj % 2 == 0:
                        nc.scalar.copy(pt_sb[:, 0:nb, :], ps_t4[:, 0:nb, :])
                    else:
                        nc.vector.tensor_copy(pt_sb[:, 0:nb, :],
                                              ps_t4[:, 0:nb, :])
                    for b in range(nb):
                        nkb = ncj * 4 + b
                        nc.tensor.matmul(out_ps, pt_sb[:, b, :],
                                         v_sb[:, nkb],
                                         start=(nkb == 0),
                                         stop=(nkb == last_nkb))
                lsum = cpool.tile([128, 1], F32, tag="lsum", bufs=4,
                                  name=f"lsum_{i}_{h}")
                nc.vector.reduce_sum(lsum[:], lpartss[h][:, 0:nparts[h]],
                                     axis=AX)
                linv = cpool.tile([128, 1], F32, tag="linv", bufs=4,
                                  name=f"linv_{i}_{h}")
                nc.vector.reciprocal(linv[:], lsum[:])
                out_pss.append(out_ps)
                linvs.append(linv)

            attn16 = cpool.tile([128, HPC, 128], F16, tag="attn16", bufs=3,
                                name=f"attn16_{i}")
            for h in range(HPC):
                at = cpool.tile([128, 128], F16, tag="at", bufs=2,
                                name=f"at_{i}_{h}")
                nc.vector.tensor_scalar_mul(at[:], out_pss[h], linvs[h][:])
                pat = psum.tile([128, 128], F16, tag="t128", bufs=3,
                                name=f"pat_{i}_{h}")
                nc.tensor.transpose(pat[:], at[:], ident16[:])
                nc.vector.tensor_copy(attn16[:, h], pat[:])
            return attn16

        # O proj emitted one stage later so its matmuls (whose attn16 is
        # long ready) fill the next block's exp wait on the PE queue
        def oproj(i, attn16):
            for nh_ in range(4):
                ns = slice(nh_ * 512, (nh_ + 1) * 512)
                po = psum.tile([128, 512], F32, tag="pod", bufs=1,
                               name=f"po_{i}_{nh_}")
                nc.tensor.matmul(po[:], attn16[:, 0], woh_sb[:, 0, ns],
                                 start=True, stop=False)
                nc.tensor.matmul(po[:], attn16[:, 1], woh_sb[:, 1, ns],
                                 start=False, stop=True)
                ob = dpool.tile([128, 512], F16, tag="ob", bufs=2,
                                name=f"ob_{i}_{nh_}")
                if nh_ % 2 == 0:
                    nc.vector.tensor_copy(ob[:], po[:])
                else:
                    nc.scalar.copy(ob[:], po[:])
                nc.sync.dma_start(out_d[i * 128:(i + 1) * 128, ns], ob[:])

        # ============== interleaved schedule ==============
        # quarter m+1 is emitted before attention batch m so the in-order
        # PE queue has projection matmuls to chew on while batch m's
        # softmax chains drain; the S1/S2 pipeline bridges batch bounds.
        prev = [None]      # pending stage1 state
        pend_o = [None]    # pending (i, attn16) for deferred O proj

        def emit_c(i):
            st = stage1(i)
            if pend_o[0] is not None:
                oproj(*pend_o[0])
                pend_o[0] = None
            if prev[0] is not None:
                pi = prev[0]["i"]
                a16 = stage2(prev[0])
                pend_o[0] = (pi, a16)
            prev[0] = st

        quarter(0)
        for mq_ in range(1, 4):
            quarter(mq_)
            for i in range(4 * (mq_ - 1), 4 * mq_):
                emit_c(i)
        for i in range(12, 16):
            emit_c(i)
        if pend_o[0] is not None:
            oproj(*pend_o[0])
        pi = prev[0]["i"]
        a16 = stage2(prev[0])
        oproj(pi, a16)

    nc.compile()
    return nc


def _f8(a):
    return np.asarray(a, np.float32).astype(E4M3)


def kernel(**inputs):
    x = np.asarray(inputs["x"], np.float32)
    cos = np.asarray(inputs["cos"], np.float32)
    sin = np.asarray(inputs["sin"], np.float32)
    am = np.asarray(inputs["attention_mask"]).reshape(S, S).astype(bool)
    wq = np.asarray(inputs["wq"], np.float32)
    bq = np.asarray(inputs["bq"], np.float32)
    wk = np.asarray(inputs["wk"], np.float32)
    bk = np.asarray(inputs["bk"], np.float32)
    wv = np.asarray(inputs["wv"], np.float32)
    bv = np.asarray(inputs["bv"], np.float32)
    wo = np.asarray(inputs["wo"], np.float32)
    bo = np.asarray(inputs["bo"], np.float32)
    qn = np.asarray(inputs["q_norm_w"], np.float32)
    kn = np.asarray(inputs["k_norm_w"], np.float32)

    assert x.shape == (1, S, H)
    is_causal = bool(
        (am == np.triu(np.ones((S, S), dtype=bool), k=1)).all())

    key = is_causal
    if key not in _prog_cache:
        _prog_cache[key] = _build(is_causal)
    nc = _prog_cache[key]

    xT = np.ascontiguousarray(x[0].T)
    xh16 = (xT * SC_X).astype(np.float16)
    xl = xT - xh16.astype(np.float32) / SC_X
    xdr = np.stack([_f8(xT), _f8(xl * SC_XL)], axis=1)   # [H, 2, S]
    # swizzle to [group, quarter, p, ...] flat layouts
    xh16 = np.ascontiguousarray(
        xh16.reshape(4, 4, 128, 4, 512).transpose(0, 3, 2, 1, 4)
        .reshape(4, 4, 128, 4 * 512))
    xdr = np.ascontiguousarray(
        xdr.reshape(4, 4, 128, 2, 4, 512).transpose(0, 4, 2, 1, 3, 5)
        .reshape(4, 4, 128, 4 * 2 * 512))

    cosT = cos.T
    sinT = sin.T
    rolled_q = np.roll(qn, -64)     # rot(q*qn)[i] = rot(q)[i] * qn[(i+64)%128]
    rolled_k = np.roll(kn, -64)
    halfneg = np.concatenate([-np.ones(64, np.float32),
                              np.ones(64, np.float32)])
    cosq = np.ascontiguousarray(SC_R * cosT * qn[:, None])
    cosk = np.ascontiguousarray(SC_R * cosT * kn[:, None])
    # sin tables rotated 64 rows so the rope half-swap muls are
    # partition-aligned on device
    sinq = np.ascontiguousarray(
        np.roll(SC_R * sinT * (rolled_q * halfneg)[:, None], 64, axis=0))
    sink = np.ascontiguousarray(
        np.roll(SC_R * sinT * (rolled_k * halfneg)[:, None], 64, axis=0))
    if not is_causal:
        maskadd = np.where(am, np.float32(NEG), np.float32(0.0))

    def wsplit(w):
        wh16 = (w * SC_W).astype(np.float16)
        wl = w - wh16.astype(np.float32) / SC_W
        wdr = np.ascontiguousarray(
            np.stack([_f8(wl * SC_WL), _f8(w * SC_WH8)], axis=1))
        return np.ascontiguousarray(wh16), wdr

    in_maps = []
    for c in range(NCORES):
        fq = slice(c * FQ, (c + 1) * FQ)
        g = c // 2
        fk = slice(g * HD, (g + 1) * HD)
        wqh, wqdr = wsplit(wq[:, fq])
        wkh, wkdr = wsplit(wk[:, fk])
        m = dict(
            xh=xh16, xdr=xdr,
            wqh=wqh, wqdr=wqdr, wkh=wkh, wkdr=wkdr,
            wvh=np.ascontiguousarray((wv[:, fk] * SC_W).astype(np.float16)),
            woh=np.ascontiguousarray(wo[fq, :].astype(np.float16)),
            cosq=cosq, sinq=sinq, cosk=cosk, sink=sink,
            bqt=np.ascontiguousarray(bq[fq].reshape(HPC, HD).T),
            bkt=np.ascontiguousarray(bk[fk].reshape(1, HD).T),
            bvt=np.ascontiguousarray(bv[fk].reshape(1, HD).T),
        )
        if not is_causal:
            m["maskadd"] = maskadd
        in_maps.append(m)

    res = bass_utils.run_bass_kernel_spmd(nc, in_maps,
                                          core_ids=list(range(NCORES)))
    acc = np.zeros((S, H), np.float64)
    for c in range(NCORES):
        acc += res.results[c]["out"].astype(np.float64)
    out = (acc + bo[None, :]).astype(np.float32)
    return out.reshape(1, S, H)
